# revision 29
# baseline (speedup 1.0000x reference)
"""EntropyGuidedAttention on 8 Trainium2 NeuronCores.

Sharding: data-parallel over batch (2) x tensor-parallel over heads (16/4=4
per core).  Core c handles batch c//4 and heads [4*(c%4), 4*(c%4)+4).
qkv is column-parallel, out_proj row-parallel; the per-batch sum over the
4 head-group partials (an AllReduce in classic TP) is done on the host as
part of unsharding, along with + b_out.

Device math per core (weights pre-folded on host):
  xn   = (x - mu) * rsqrt(var + 1e-6)                  (ln_g/ln_b folded into W)
  qT,kT = Wq'/Wk' blocks @ xn^T   (Wq' includes scale/TEMP = 1.25)
  v     = xn @ Wv'^T ; gate = 1/(1+exp(-z-be)) clamped [.1,2] applied to v
  St    = kT^T q (scores transposed, [k, q] layout), Pt = exp(St) * causal
  PVZ   = [ones | v'^T] @ Pt accumulated over kt: partitions 0:64 hold Z
          replicas, 64:128 hold numT -- one fused matmul per (kt, head)
  OT    = numT * recip(Z)   (Z pre-replicated across 64 partitions)
  out_p = sum_p OT_p^T @ Wo_p     (Wo includes the 0.1 output scale)

Activation-table discipline: the LN/QKV phase uses only {Sqrt, Identity,
Copy} (sqrt table), attention only {Exp} -- exactly two table loads.  The
entropy-gate sigmoid is a single batched Exp at the attention boundary
(z columns staged during the V pass), with 1/(1+t) + clamp on DVE, then
folded into the bf16 V halves of the PVZ stationaries.

Attention uses 256-wide q chunks so the score psum (2 banks) can be
double-buffered next to the 4 PVZ accumulators: the PE never waits on the
exp of the previous chunk.  Out-projection matmuls are interleaved into
the next chunk's St/PVZ stream (the attention phase is exp-paced, so they
ride in the PE bubbles), and output DMA streams during later chunks.
"""
import contextlib

import numpy as np

import concourse.bacc as bacc
import concourse.tile as tile
from concourse import mybir
from concourse.bass_utils import run_bass_kernel_spmd

F32 = mybir.dt.float32
F32R = mybir.dt.float32r
BF16 = mybir.dt.bfloat16
AF = mybir.ActivationFunctionType
ALU = mybir.AluOpType

H, NH, HD = 1024, 16, 64
B, S = 2, 2048
NCORES = 8
HPC = 4            # heads per core
NPAIR = 2          # head pairs per core
ST = S // 128      # 16 s-tiles
KC = H // 128      # 8 contraction chunks
QC = S // 512      # 4 ln/qkv groups of 512
AC = S // 512      # attention q chunks


def _build_nc():
    nc = bacc.Bacc("TRN2", target_bir_lowering=False, debug=False,
                   num_devices=NCORES)

    x_d = nc.dram_tensor("x", [S, H], F32, kind="ExternalInput")
    wqk_d = nc.dram_tensor("wqk", [128, KC * 512], F32, kind="ExternalInput")
    wvg_d = nc.dram_tensor("wvg", [128, KC * 258], F32, kind="ExternalInput")
    wo_d = nc.dram_tensor("wo", [128, 2 * H], F32, kind="ExternalInput")
    qkb_d = nc.dram_tensor("qkb", [128, 4], F32, kind="ExternalInput")
    vb_d = nc.dram_tensor("vb", [128, 256], F32, kind="ExternalInput")
    negentb_d = nc.dram_tensor("negentb", [128, 1], F32, kind="ExternalInput")
    ident_d = nc.dram_tensor("ident", [128, 128], F32, kind="ExternalInput")
    umask_d = nc.dram_tensor("umask", [128, HPC * 128], F32,
                             kind="ExternalInput")
    out_d = nc.dram_tensor("out_part", [S, H], F32, kind="ExternalOutput")

    with tile.TileContext(nc) as tc, contextlib.ExitStack() as ctx:
        consts = ctx.enter_context(tc.tile_pool(name="consts", bufs=1))
        qk_pool = ctx.enter_context(tc.tile_pool(name="qk", bufs=1))
        vgz_pool = ctx.enter_context(tc.tile_pool(name="vgz", bufs=1))
        ot_pool = ctx.enter_context(tc.tile_pool(name="ot", bufs=1))

        # ---- weights: x tiles own the sync queue; everything else rides
        # the scalar hwdge queue, ordered by first use ----
        ident = consts.tile([128, 128], F32R)
        nc.sync.dma_start(out=ident, in_=ident_d[:, :].bitcast(F32R))
        wqk = consts.tile([128, KC, 512], F32R)
        qkb = consts.tile([128, 4], F32)
        vb = consts.tile([128, 256], F32)
        negentb = consts.tile([128, 1], F32)
        wvg = consts.tile([128, KC, 258], F32R)
        umaskf = consts.tile([128, HPC, 128], F32)
        wo = consts.tile([128, 2, H], F32R)
        umask = consts.tile([128, HPC, 128], BF16)

        nc.scalar.dma_start(
            out=wqk, in_=wqk_d.rearrange("p (c m) -> p c m", c=KC).bitcast(F32R))
        nc.scalar.dma_start(out=qkb, in_=qkb_d[:, :])
        nc.scalar.dma_start(out=vb, in_=vb_d[:, :])
        nc.scalar.dma_start(out=negentb, in_=negentb_d[:, :])
        zstage = consts.tile([128, ST], F32)
        gst = consts.tile([128, ST], F32)
        eps = consts.tile([128, 1], F32)
        nc.vector.memset(eps, 1e-6)

        qk_big = qk_pool.tile([128, 4, S], F32R)      # qp0 qp1 kp0 kp1
        # per (s-tile, head): [ones(64) | v(64)] -> fused PV+Z stationary
        vgz = vgz_pool.tile([128, ST, HPC, 128], BF16)
        ot_big = ot_pool.tile([128, NPAIR, S], F32R)  # O^T (pair, d) x q

        # ---- phases 1-4: per group g of 4 s-tiles: layernorm + transpose
        # -> xnT chunk, then QKV-T and V for those columns.  Work is split
        # DVE/ACT to keep both under the PE time ----
        with tc.tile_pool(name="ln", bufs=4) as ln_pool, \
             tc.tile_pool(name="stats", bufs=4) as stats_pool, \
             tc.tile_pool(name="xnt", bufs=2) as xnt_pool, \
             tc.tile_pool(name="pst", bufs=4, space="PSUM") as pst, \
             tc.tile_pool(name="psq", bufs=2, space="PSUM") as psq, \
             tc.tile_pool(name="psv", bufs=2, space="PSUM") as psv:

            for g in range(QC):
                for st in range(4 * g, 4 * g + 4):
                    nc.gpsimd.memset(vgz[:, st, :, 0:64], 1.0)

                xnt = xnt_pool.tile([128, KC, 512], F32R, tag="xnt")
                for r in range(4):
                    st = 4 * g + r
                    xt = ln_pool.tile([128, H], F32, tag="x")
                    nc.sync.dma_start(out=xt,
                                      in_=x_d[st * 128:(st + 1) * 128, :])
                    if g == 0 and r == 3:
                        # weights ride behind the first four x tiles in the
                        # global DMA order; wqk lands just before QKV(g=0)
                        nc.scalar.dma_start(
                            out=wqk, in_=wqk_d.rearrange(
                                "p (c m) -> p c m", c=KC).bitcast(F32R))
                        nc.scalar.dma_start(out=qkb, in_=qkb_d[:, :])
                        nc.scalar.dma_start(out=vb, in_=vb_d[:, :])
                        nc.scalar.dma_start(out=negentb, in_=negentb_d[:, :])
                    stats = stats_pool.tile([128, 2, 6], F32, tag="bn")
                    nc.vector.bn_stats(out=stats[:, 0, :], in_=xt[:, 0:512])
                    nc.vector.bn_stats(out=stats[:, 1, :], in_=xt[:, 512:1024])
                    mv = stats_pool.tile([128, 2], F32, tag="mv")
                    nc.vector.bn_aggr(out=mv, in_=stats)
                    rstd = stats_pool.tile([128, 1], F32, tag="rstd")
                    nc.scalar.activation(out=rstd, in_=mv[:, 1:2],
                                         func=AF.Sqrt, bias=eps, scale=1.0)
                    nc.vector.reciprocal(out=rstd, in_=rstd)
                    xn = ln_pool.tile([128, H], F32R, tag="xn")
                    if r % 2 == 0:
                        nc.vector.tensor_scalar(out=xn, in0=xt,
                                                scalar1=mv[:, 0:1],
                                                scalar2=rstd,
                                                op0=ALU.subtract, op1=ALU.mult)
                    else:
                        # xn = x*rstd + (-mu*rstd) on the scalar engine
                        nmr = stats_pool.tile([128, 1], F32, tag="nmr")
                        nc.vector.tensor_scalar(out=nmr, in0=mv[:, 0:1],
                                                scalar1=rstd, scalar2=-1.0,
                                                op0=ALU.mult, op1=ALU.mult)
                        nc.scalar.activation(out=xn, in_=xt,
                                             func=AF.Identity,
                                             bias=nmr, scale=rstd)
                    # transpose 8 h-chunks; 4 per psum bank; psum->sbuf
                    # copies split across vector and scalar engines
                    for half in range(2):
                        ptr = pst.tile([128, 4, 128], F32R, tag="tr")
                        for j in range(4):
                            c = half * 4 + j
                            nc.tensor.transpose(ptr[:, j, :],
                                                xn[:, c * 128:(c + 1) * 128],
                                                ident)
                        dst = xnt[:, half * 4:half * 4 + 4,
                                  r * 128:(r + 1) * 128]
                        if half == 0:
                            nc.vector.tensor_copy(dst, ptr)
                        else:
                            nc.scalar.copy(dst, ptr)

                if g == 0:
                    nc.scalar.dma_start(
                        out=wvg, in_=wvg_d.rearrange(
                            "p (c m) -> p c m", c=KC).bitcast(F32R))
                elif g == 1:
                    nc.scalar.dma_start(
                        out=umaskf, in_=umask_d.rearrange(
                            "p (u m) -> p u m", u=HPC))
                    nc.scalar.dma_start(
                        out=wo, in_=wo_d.rearrange(
                            "p (c m) -> p c m", c=2).bitcast(F32R))
                # QKV-T for this 512-wide chunk of S
                for mb in range(4):
                    pq = psq.tile([128, 512], F32, tag="q")
                    for c in range(KC):
                        nc.tensor.matmul(pq[:, :],
                                         wqk[:, c, mb * 128:(mb + 1) * 128],
                                         xnt[:, c, :],
                                         start=(c == 0), stop=(c == KC - 1))
                    nc.vector.tensor_scalar(
                        out=qk_big[:, mb, g * 512:(g + 1) * 512],
                        in0=pq[:, :], scalar1=qkb[:, mb:mb + 1], scalar2=None,
                        op0=ALU.add)

                # V (ungated) + entropy-z staging for these 4 s-tiles
                for r in range(4):
                    st = 4 * g + r
                    pv = psv.tile([128, 258], F32, tag="v")
                    for c in range(KC):
                        nc.tensor.matmul(pv[:, :],
                                         xnt[:, c, r * 128:(r + 1) * 128],
                                         wvg[:, c, :],
                                         start=(c == 0), stop=(c == KC - 1))
                    nc.vector.tensor_copy(zstage[:, st:st + 1],
                                          pv[:, 256:257])
                    vtmp = ln_pool.tile([128, 256], F32, tag="vtmp")
                    nc.vector.tensor_add(vtmp, pv[:, 0:256], vb)
                    nc.scalar.copy(vgz[:, st, :, 64:128], vtmp)

        # ---- phase 5: attention (St/exp/PVZ pipeline over 256-wide q
        # chunks; st4 double-buffered so the PE never waits on exp), with
        # the gate fold-in up front and out-projection interleaved ----
        with tc.tile_pool(name="pt", bufs=3) as pt_pool, \
             tc.tile_pool(name="rz", bufs=2) as rz_pool, \
             tc.tile_pool(name="ost", bufs=4) as ost_pool, \
             tc.tile_pool(name="ps_att", bufs=1, space="PSUM") as ps_att:

            nc.vector.tensor_copy(umask, umaskf)
            # gate = clip(1/(1+exp(-z-be)), .1, 2) for all 16 s-tiles in one
            # go, then fold into the bf16 V halves of the PVZ stationaries
            nc.scalar.activation(out=gst, in_=zstage, func=AF.Exp,
                                 bias=negentb, scale=-1.0)
            nc.vector.tensor_scalar(out=gst, in0=gst, scalar1=1.0,
                                    scalar2=None, op0=ALU.add)
            gtmp = ost_pool.tile([128, ST], F32, tag="gtmp")
            nc.vector.reciprocal_approx_fast(out=gtmp, in_=gst)
            nc.vector.tensor_scalar(out=gst, in0=gtmp, scalar1=0.1,
                                    scalar2=2.0, op0=ALU.max, op1=ALU.min)
            for st in range(ST):
                nc.vector.tensor_scalar(
                    out=vgz[:, st, :, 64:128], in0=vgz[:, st, :, 64:128],
                    scalar1=gst[:, st:st + 1], scalar2=None, op0=ALU.mult)

            def emit_st_exp(qc, kt, w):
                # one half-wave: heads {2w, 2w+1}; A/B staggering keeps the
                # PE ahead of the (slower) exp stream without double-buffers
                off = max(kt * 128 - qc * 512, 0)
                moff = min(off, 256)   # f32r wants >=256 moving rows
                st2 = ps_att.tile([128, 2, 512], F32,
                                  name=f"st{w}_{qc}_{kt}", tag=f"st{w}")
                for i in range(2):
                    h = 2 * w + i
                    p, a = h // 2, h % 2
                    nc.tensor.matmul(
                        st2[:, i, moff:],
                        qk_big[64 * a:64 * a + 64, 2 + p,
                               kt * 128:(kt + 1) * 128],
                        qk_big[64 * a:64 * a + 64, p,
                               qc * 512 + moff:(qc + 1) * 512],
                        start=True, stop=True,
                        tile_position=(64 * a, 0))
                pt2 = pt_pool.tile([128, 2, 512], BF16,
                                   name=f"pt{w}_{qc}_{kt}", tag=f"pt{w}")
                nc.scalar.activation(out=pt2[:, :, off:], in_=st2[:, :, off:],
                                     func=AF.Exp)
                if kt * 128 >= qc * 512:   # diagonal k-tile
                    nc.vector.tensor_mul(pt2[:, :, off:off + 128],
                                         pt2[:, :, off:off + 128],
                                         umask[:, 0:2, :])
                return pt2

            def emit_pvz(pvzt, pt2, qc, kt, nkt, w):
                first, last = kt == 0, kt == nkt - 1
                off = max(kt * 128 - qc * 512, 0)
                for i in range(2):
                    h = 2 * w + i
                    nc.tensor.matmul(
                        pvzt[h][:, off:],
                        vgz[:, kt, h, :],
                        pt2[:, i, off:],
                        start=first, stop=last)

            pending = []   # s-tile out-projection units not yet emitted

            def emit_po(st):
                po = ps_att.tile([128, 2, 512], F32, name=f"po_{st}",
                                 tag=f"st{st % 2}")
                for n in range(2):
                    for p in range(NPAIR):
                        nc.tensor.matmul(
                            po[:, n, :],
                            ot_big[:, p, st * 128:(st + 1) * 128],
                            wo[:, p, n * 512:(n + 1) * 512],
                            start=(p == 0), stop=(p == NPAIR - 1))
                ob = ost_pool.tile([128, 1024], F32, tag="ob")
                nc.vector.tensor_copy(ob, po)
                nc.sync.dma_start(
                    out=out_d[st * 128:(st + 1) * 128, :], in_=ob[:, :])

            for qc in range(AC):
                nkt = 4 * qc + 4
                pvzt = [ps_att.tile([128, 512], F32, name=f"pvz{h}_{qc}",
                                    tag=f"pvz{h}")
                        for h in range(HPC)]
                hist = []
                for kt in range(nkt):
                    hist.append((emit_st_exp(qc, kt, 0),
                                 emit_st_exp(qc, kt, 1)))
                    # two-iteration St runway: the PVZ stream (whose psum
                    # banks wait on the previous chunk's normalize at kt=0)
                    # trails two k-tiles behind the St/exp stream
                    if kt >= 2:
                        pk = kt - 2
                        emit_pvz(pvzt, hist[pk][0], qc, pk, nkt, 0)
                        emit_pvz(pvzt, hist[pk][1], qc, pk, nkt, 1)
                    # previous chunks' out-projection rides the PE bubbles
                    # of this exp-paced stream, one unit per odd iteration
                    if pending and kt % 2 == 1 and 3 <= kt < nkt - 1:
                        emit_po(pending.pop(0))
                for pk in range(max(nkt - 2, 0), nkt):
                    emit_pvz(pvzt, hist[pk][0], qc, pk, nkt, 0)
                    emit_pvz(pvzt, hist[pk][1], qc, pk, nkt, 1)

                # normalize: OT = numT * recip(Z); Z replicas on partitions
                # 0:64 of each pvz tile, numT on 64:128
                for h in range(HPC):
                    p, a = h // 2, h % 2
                    rz = rz_pool.tile([64, 512], F32, tag=f"rz{h % 2}")
                    nc.vector.reciprocal_approx_fast(out=rz,
                                                     in_=pvzt[h][0:64, :])
                    nc.vector.tensor_mul(
                        ot_big[64 * a:64 * a + 64, p,
                               qc * 512:(qc + 1) * 512],
                        pvzt[h][64:128, :], rz)
                pending.extend(4 * qc + half for half in range(4))




            for unit in pending:
                emit_po(unit)

    nc.compile()
    return nc


_NC = None


def _get_nc():
    global _NC
    if _NC is None:
        _NC = _build_nc()
    return _NC


def _in_maps(inputs):
    x = np.ascontiguousarray(np.asarray(inputs["x"], np.float32))
    ln_g = np.asarray(inputs["ln_g"], np.float32)
    ln_b = np.asarray(inputs["ln_b"], np.float32)
    w_qkv = np.asarray(inputs["w_qkv"], np.float32)
    b_qkv = np.asarray(inputs["b_qkv"], np.float32)
    w_ent = np.asarray(inputs["w_ent"], np.float32)
    b_ent = np.asarray(inputs["b_ent"], np.float32)

    qmul = np.float32((1.0 / np.sqrt(np.float32(HD))) / 0.1)

    wq = w_qkv[:H] * ln_g[None, :]
    wk = w_qkv[H:2 * H] * ln_g[None, :]
    wv = w_qkv[2 * H:] * ln_g[None, :]
    bq = (b_qkv[:H] + wq @ ln_b) * qmul
    bk = b_qkv[H:2 * H] + wk @ ln_b
    bv = b_qkv[2 * H:] + wv @ ln_b
    wq = wq * qmul
    went = (w_ent * ln_g[None, :])[0]
    bent = np.float32(b_ent[0] + w_ent[0] @ ln_b)
    w_out = np.asarray(inputs["w_out"], np.float32)

    ident = np.eye(128, dtype=np.float32)
    umask = np.ascontiguousarray(np.broadcast_to(
        np.triu(np.ones((128, 128), np.float32))[:, None, :],
        (128, HPC, 128)).reshape(128, HPC * 128))

    in_maps = []
    for c in range(NCORES):
        b, g = divmod(c, NCORES // B)
        r = slice(g * HPC * HD, (g + 1) * HPC * HD)
        wqkT = np.concatenate([wq[r], wk[r]], axis=0).T       # [H, 512]
        wqk_r = np.ascontiguousarray(
            wqkT.reshape(KC, 128, 512).transpose(1, 0, 2).reshape(128, -1))
        wvgT = np.concatenate([wv[r], went[None, :],
                               np.zeros((1, H), np.float32)], axis=0).T
        wvg_r = np.ascontiguousarray(
            wvgT.reshape(KC, 128, 258).transpose(1, 0, 2).reshape(128, -1))
        woT = (0.1 * w_out[:, r]).T                           # [256, H]
        wo_r = np.ascontiguousarray(
            woT.reshape(2, 128, H).transpose(1, 0, 2).reshape(128, -1))
        qkb_r = np.ascontiguousarray(
            np.concatenate([bq[r], bk[r]]).reshape(4, 128).T)
        vb_r = np.ascontiguousarray(
            np.broadcast_to(bv[r][None, :], (128, 256)))
        in_maps.append({
            "x": x[b], "wqk": wqk_r, "wvg": wvg_r, "wo": wo_r,
            "qkb": qkb_r, "vb": vb_r,
            "negentb": np.full((128, 1), -bent, np.float32),
            "ident": ident, "umask": umask,
        })
    return in_maps


def _unshard(inputs, results):
    b_out = np.asarray(inputs["b_out"], np.float32)
    outs = []
    for b in range(B):
        g0 = b * (NCORES // B)
        acc = results[g0]["out_part"].astype(np.float32)
        for g in range(g0 + 1, g0 + NCORES // B):
            acc = acc + results[g]["out_part"]
        outs.append(acc + 0.1 * b_out[None, :])
    return np.stack(outs)


def run(inputs, **kw):
    nc = _get_nc()
    res = run_bass_kernel_spmd(nc, _in_maps(inputs),
                               core_ids=list(range(NCORES)), **kw)
    return _unshard(inputs, res.results), res


def kernel(**inputs) -> np.ndarray:
    out, _ = run(inputs)
    return out


# revision 30
# speedup vs baseline: 1.1057x; 1.1057x over previous
"""EntropyGuidedAttention on 8 Trainium2 NeuronCores.

Sharding: data-parallel over batch (2) x tensor-parallel over heads (16/4=4
per core).  Core c handles batch c//4 and heads [4*(c%4), 4*(c%4)+4).
qkv is column-parallel, out_proj row-parallel; the per-batch sum over the
4 head-group partials (an AllReduce in classic TP) is done on the host as
part of unsharding, along with + b_out.

Device math per core (weights pre-folded on host):
  xn   = (x - mu) * rsqrt(var + 1e-6)                  (ln_g/ln_b folded into W)
  qT,kT = Wq'/Wk' blocks @ xn^T   (Wq' includes scale/TEMP = 1.25)
  v     = xn @ Wv'^T ; gate = 1/(1+exp(-z-be)) clamped [.1,2] applied to v
  St    = kT^T q (scores transposed, [k, q] layout), Pt = exp(St) * causal
  PVZ   = [ones | v'^T] @ Pt accumulated over kt: partitions 0:64 hold Z
          replicas, 64:128 hold numT -- one fused matmul per (kt, head)
  OT    = numT * recip(Z)   (Z pre-replicated across 64 partitions)
  out_p = sum_p OT_p^T @ Wo_p     (Wo includes the 0.1 output scale)

Activation-table discipline: the LN/QKV phase uses only {Sqrt, Identity,
Copy} (sqrt table), attention only {Exp} -- exactly two table loads.  The
entropy-gate sigmoid is a single batched Exp at the attention boundary
(z columns staged during the V pass), with 1/(1+t) + clamp on DVE, then
folded into the bf16 V halves of the PVZ stationaries.

Attention uses 256-wide q chunks so the score psum (2 banks) can be
double-buffered next to the 4 PVZ accumulators: the PE never waits on the
exp of the previous chunk.  Out-projection matmuls are interleaved into
the next chunk's St/PVZ stream (the attention phase is exp-paced, so they
ride in the PE bubbles), and output DMA streams during later chunks.
"""
import contextlib

import numpy as np

import concourse.bacc as bacc
import concourse.tile as tile
from concourse import mybir
from concourse.bass_utils import run_bass_kernel_spmd

F32 = mybir.dt.float32
F32R = mybir.dt.float32r
BF16 = mybir.dt.bfloat16
AF = mybir.ActivationFunctionType
ALU = mybir.AluOpType

H, NH, HD = 1024, 16, 64
B, S = 2, 2048
NCORES = 8
HPC = 4            # heads per core
NPAIR = 2          # head pairs per core
ST = S // 128      # 16 s-tiles
KC = H // 128      # 8 contraction chunks
QC = S // 512      # 4 ln/qkv groups of 512
AC = S // 512      # attention q chunks


def _build_nc():
    nc = bacc.Bacc("TRN2", target_bir_lowering=False, debug=False,
                   num_devices=NCORES)

    x_d = nc.dram_tensor("x", [S, H], F32, kind="ExternalInput")
    wqk_d = nc.dram_tensor("wqk", [128, KC * 512], F32, kind="ExternalInput")
    wvg_d = nc.dram_tensor("wvg", [128, KC * 258], F32, kind="ExternalInput")
    wo_d = nc.dram_tensor("wo", [128, 2 * H], F32, kind="ExternalInput")
    qkb_d = nc.dram_tensor("qkb", [128, 4], F32, kind="ExternalInput")
    vb_d = nc.dram_tensor("vb", [128, 256], F32, kind="ExternalInput")
    negentb_d = nc.dram_tensor("negentb", [128, 1], F32, kind="ExternalInput")
    ident_d = nc.dram_tensor("ident", [128, 128], F32, kind="ExternalInput")
    umask_d = nc.dram_tensor("umask", [128, HPC * 128], F32,
                             kind="ExternalInput")
    out_d = nc.dram_tensor("out_part", [S, H], F32, kind="ExternalOutput")

    with tile.TileContext(nc) as tc, contextlib.ExitStack() as ctx:
        consts = ctx.enter_context(tc.tile_pool(name="consts", bufs=1))
        qk_pool = ctx.enter_context(tc.tile_pool(name="qk", bufs=1))
        vgz_pool = ctx.enter_context(tc.tile_pool(name="vgz", bufs=1))
        ot_pool = ctx.enter_context(tc.tile_pool(name="ot", bufs=1))

        # ---- weights: x tiles own the sync queue; everything else rides
        # the scalar hwdge queue, ordered by first use ----
        ident = consts.tile([128, 128], F32R)
        nc.sync.dma_start(out=ident, in_=ident_d[:, :].bitcast(F32R))
        wqk = consts.tile([128, KC, 512], F32R)
        qkb = consts.tile([128, 4], F32)
        vb = consts.tile([128, 256], F32)
        negentb = consts.tile([128, 1], F32)
        wvg = consts.tile([128, KC, 258], F32R)
        umaskf = consts.tile([128, HPC, 128], F32)
        wo = consts.tile([128, 2, H], F32R)
        umask = consts.tile([128, HPC, 128], BF16)

        nc.scalar.dma_start(
            out=wqk, in_=wqk_d.rearrange("p (c m) -> p c m", c=KC).bitcast(F32R))
        nc.scalar.dma_start(out=qkb, in_=qkb_d[:, :])
        nc.scalar.dma_start(
            out=wvg, in_=wvg_d.rearrange("p (c m) -> p c m", c=KC).bitcast(F32R))
        nc.scalar.dma_start(out=vb, in_=vb_d[:, :])
        nc.scalar.dma_start(out=negentb, in_=negentb_d[:, :])
        nc.scalar.dma_start(
            out=umaskf, in_=umask_d.rearrange("p (u m) -> p u m", u=HPC))
        nc.scalar.dma_start(
            out=wo, in_=wo_d.rearrange("p (c m) -> p c m", c=2).bitcast(F32R))
        zstage = consts.tile([128, ST], F32)
        gst = consts.tile([128, ST], F32)
        eps = consts.tile([128, 1], F32)
        nc.vector.memset(eps, 1e-6)

        qk_big = qk_pool.tile([128, 4, S], F32R)      # qp0 qp1 kp0 kp1
        # per (s-tile, head): [ones(64) | v(64)] -> fused PV+Z stationary
        vgz = vgz_pool.tile([128, ST, HPC, 128], BF16)
        ot_big = ot_pool.tile([128, NPAIR, S], F32R)  # O^T (pair, d) x q

        # ---- phases 1-4: per group g of 4 s-tiles: layernorm + transpose
        # -> xnT chunk, then QKV-T and V for those columns.  Work is split
        # DVE/ACT to keep both under the PE time ----
        with tc.tile_pool(name="ln", bufs=4) as ln_pool, \
             tc.tile_pool(name="stats", bufs=4) as stats_pool, \
             tc.tile_pool(name="xnt", bufs=2) as xnt_pool, \
             tc.tile_pool(name="pst", bufs=4, space="PSUM") as pst, \
             tc.tile_pool(name="psq", bufs=2, space="PSUM") as psq, \
             tc.tile_pool(name="psv", bufs=2, space="PSUM") as psv:

            for g in range(QC):
                for st in range(4 * g, 4 * g + 4):
                    nc.gpsimd.memset(vgz[:, st, :, 0:64], 1.0)

                xnt = xnt_pool.tile([128, KC, 512], F32R, tag="xnt")
                for r in range(4):
                    st = 4 * g + r
                    xt = ln_pool.tile([128, H], F32, tag="x")
                    nc.sync.dma_start(out=xt,
                                      in_=x_d[st * 128:(st + 1) * 128, :])
                    stats = stats_pool.tile([128, 2, 6], F32, tag="bn")
                    nc.vector.bn_stats(out=stats[:, 0, :], in_=xt[:, 0:512])
                    nc.vector.bn_stats(out=stats[:, 1, :], in_=xt[:, 512:1024])
                    mv = stats_pool.tile([128, 2], F32, tag="mv")
                    nc.vector.bn_aggr(out=mv, in_=stats)
                    rstd = stats_pool.tile([128, 1], F32, tag="rstd")
                    nc.scalar.activation(out=rstd, in_=mv[:, 1:2],
                                         func=AF.Sqrt, bias=eps, scale=1.0)
                    nc.vector.reciprocal(out=rstd, in_=rstd)
                    xn = ln_pool.tile([128, H], F32R, tag="xn")
                    if r % 2 == 0:
                        nc.vector.tensor_scalar(out=xn, in0=xt,
                                                scalar1=mv[:, 0:1],
                                                scalar2=rstd,
                                                op0=ALU.subtract, op1=ALU.mult)
                    else:
                        # xn = x*rstd + (-mu*rstd) on the scalar engine
                        nmr = stats_pool.tile([128, 1], F32, tag="nmr")
                        nc.vector.tensor_scalar(out=nmr, in0=mv[:, 0:1],
                                                scalar1=rstd, scalar2=-1.0,
                                                op0=ALU.mult, op1=ALU.mult)
                        nc.scalar.activation(out=xn, in_=xt,
                                             func=AF.Identity,
                                             bias=nmr, scale=rstd)
                    # transpose 8 h-chunks; 4 per psum bank; psum->sbuf
                    # copies split across vector and scalar engines
                    for half in range(2):
                        ptr = pst.tile([128, 4, 128], F32R, tag="tr")
                        for j in range(4):
                            c = half * 4 + j
                            nc.tensor.transpose(ptr[:, j, :],
                                                xn[:, c * 128:(c + 1) * 128],
                                                ident)
                        dst = xnt[:, half * 4:half * 4 + 4,
                                  r * 128:(r + 1) * 128]
                        if half == 0:
                            nc.vector.tensor_copy(dst, ptr)
                        else:
                            nc.scalar.copy(dst, ptr)

                # QKV-T for this 512-wide chunk of S (bias add on ACT)
                for mb in range(4):
                    pq = psq.tile([128, 512], F32, tag="q")
                    for c in range(KC):
                        nc.tensor.matmul(pq[:, :],
                                         wqk[:, c, mb * 128:(mb + 1) * 128],
                                         xnt[:, c, :],
                                         start=(c == 0), stop=(c == KC - 1))
                    nc.vector.tensor_scalar(
                        out=qk_big[:, mb, g * 512:(g + 1) * 512],
                        in0=pq[:, :], scalar1=qkb[:, mb:mb + 1], scalar2=None,
                        op0=ALU.add)

                # V (ungated) + entropy-z staging for these 4 s-tiles
                for r in range(4):
                    st = 4 * g + r
                    pv = psv.tile([128, 258], F32, tag="v")
                    for c in range(KC):
                        nc.tensor.matmul(pv[:, :],
                                         xnt[:, c, r * 128:(r + 1) * 128],
                                         wvg[:, c, :],
                                         start=(c == 0), stop=(c == KC - 1))
                    nc.vector.tensor_copy(zstage[:, st:st + 1],
                                          pv[:, 256:257])
                    vtmp = ln_pool.tile([128, 256], F32, tag="vtmp")
                    nc.vector.tensor_add(vtmp, pv[:, 0:256], vb)
                    nc.scalar.copy(vgz[:, st, :, 64:128], vtmp)

        # ---- phase 5: attention (St/exp/PVZ pipeline over 256-wide q
        # chunks; st4 double-buffered so the PE never waits on exp), with
        # the gate fold-in up front and out-projection interleaved ----
        with tc.tile_pool(name="pt", bufs=3) as pt_pool, \
             tc.tile_pool(name="rz", bufs=2) as rz_pool, \
             tc.tile_pool(name="ost", bufs=4) as ost_pool, \
             tc.tile_pool(name="ps_att", bufs=1, space="PSUM") as ps_att:

            nc.vector.tensor_copy(umask, umaskf)
            # gate = clip(1/(1+exp(-z-be)), .1, 2) for all 16 s-tiles in one
            # go, then fold into the bf16 V halves of the PVZ stationaries
            nc.scalar.activation(out=gst, in_=zstage, func=AF.Exp,
                                 bias=negentb, scale=-1.0)
            nc.vector.tensor_scalar(out=gst, in0=gst, scalar1=1.0,
                                    scalar2=None, op0=ALU.add)
            gtmp = ost_pool.tile([128, ST], F32, tag="gtmp")
            nc.vector.reciprocal_approx_fast(out=gtmp, in_=gst)
            nc.vector.tensor_scalar(out=gst, in0=gtmp, scalar1=0.1,
                                    scalar2=2.0, op0=ALU.max, op1=ALU.min)
            for st in range(ST):
                nc.vector.tensor_scalar(
                    out=vgz[:, st, :, 64:128], in0=vgz[:, st, :, 64:128],
                    scalar1=gst[:, st:st + 1], scalar2=None, op0=ALU.mult)

            def emit_st_exp(qc, kt, w):
                # one half-wave: heads {2w, 2w+1}; A/B staggering keeps the
                # PE ahead of the (slower) exp stream without double-buffers
                off = max(kt * 128 - qc * 512, 0)
                moff = min(off, 256)   # f32r wants >=256 moving rows
                st2 = ps_att.tile([128, 2, 512], F32,
                                  name=f"st{w}_{qc}_{kt}", tag=f"st{w}")
                for i in range(2):
                    h = 2 * w + i
                    p, a = h // 2, h % 2
                    nc.tensor.matmul(
                        st2[:, i, moff:],
                        qk_big[64 * a:64 * a + 64, 2 + p,
                               kt * 128:(kt + 1) * 128],
                        qk_big[64 * a:64 * a + 64, p,
                               qc * 512 + moff:(qc + 1) * 512],
                        start=True, stop=True,
                        tile_position=(64 * a, 0))
                pt2 = pt_pool.tile([128, 2, 512], BF16,
                                   name=f"pt{w}_{qc}_{kt}", tag=f"pt{w}")
                nc.scalar.activation(out=pt2[:, :, off:], in_=st2[:, :, off:],
                                     func=AF.Exp)
                if kt * 128 >= qc * 512:   # diagonal k-tile
                    nc.vector.tensor_mul(pt2[:, :, off:off + 128],
                                         pt2[:, :, off:off + 128],
                                         umask[:, 0:2, :])
                return pt2

            def emit_pvz(pvzt, pt2, qc, kt, nkt, w):
                first, last = kt == 0, kt == nkt - 1
                off = max(kt * 128 - qc * 512, 0)
                for i in range(2):
                    h = 2 * w + i
                    nc.tensor.matmul(
                        pvzt[h][:, off:],
                        vgz[:, kt, h, :],
                        pt2[:, i, off:],
                        start=first, stop=last)

            pending = []   # s-tile out-projection units not yet emitted

            def emit_po(st):
                po = ps_att.tile([128, 2, 512], F32, name=f"po_{st}",
                                 tag=f"st{st % 2}")
                for n in range(2):
                    for p in range(NPAIR):
                        nc.tensor.matmul(
                            po[:, n, :],
                            ot_big[:, p, st * 128:(st + 1) * 128],
                            wo[:, p, n * 512:(n + 1) * 512],
                            start=(p == 0), stop=(p == NPAIR - 1))
                ob = ost_pool.tile([128, 1024], F32, tag="ob")
                nc.vector.tensor_copy(ob, po)
                nc.sync.dma_start(
                    out=out_d[st * 128:(st + 1) * 128, :], in_=ob[:, :])

            for qc in range(AC):
                nkt = 4 * qc + 4
                pvzt = [ps_att.tile([128, 512], F32, name=f"pvz{h}_{qc}",
                                    tag=f"pvz{h}")
                        for h in range(HPC)]
                hist = []
                for kt in range(nkt):
                    hist.append((emit_st_exp(qc, kt, 0),
                                 emit_st_exp(qc, kt, 1)))
                    # two-iteration St runway: the PVZ stream (whose psum
                    # banks wait on the previous chunk's normalize at kt=0)
                    # trails two k-tiles behind the St/exp stream
                    if kt >= 2:
                        pk = kt - 2
                        emit_pvz(pvzt, hist[pk][0], qc, pk, nkt, 0)
                        emit_pvz(pvzt, hist[pk][1], qc, pk, nkt, 1)
                    # previous chunks' out-projection rides the PE bubbles
                    # of this exp-paced stream, one unit per odd iteration
                    if pending and kt % 2 == 1 and 3 <= kt < nkt - 1:
                        emit_po(pending.pop(0))
                for pk in range(max(nkt - 2, 0), nkt):
                    emit_pvz(pvzt, hist[pk][0], qc, pk, nkt, 0)
                    emit_pvz(pvzt, hist[pk][1], qc, pk, nkt, 1)

                # normalize: OT = numT * recip(Z); Z replicas on partitions
                # 0:64 of each pvz tile, numT on 64:128
                for h in range(HPC):
                    p, a = h // 2, h % 2
                    rz = rz_pool.tile([64, 512], F32, tag=f"rz{h % 2}")
                    nc.vector.reciprocal_approx_fast(out=rz,
                                                     in_=pvzt[h][0:64, :])
                    nc.vector.tensor_mul(
                        ot_big[64 * a:64 * a + 64, p,
                               qc * 512:(qc + 1) * 512],
                        pvzt[h][64:128, :], rz)
                pending.extend(4 * qc + half for half in range(4))




            for unit in pending:
                emit_po(unit)

    nc.compile()
    return nc


_NC = None


def _get_nc():
    global _NC
    if _NC is None:
        _NC = _build_nc()
    return _NC


def _in_maps(inputs):
    x = np.ascontiguousarray(np.asarray(inputs["x"], np.float32))
    ln_g = np.asarray(inputs["ln_g"], np.float32)
    ln_b = np.asarray(inputs["ln_b"], np.float32)
    w_qkv = np.asarray(inputs["w_qkv"], np.float32)
    b_qkv = np.asarray(inputs["b_qkv"], np.float32)
    w_ent = np.asarray(inputs["w_ent"], np.float32)
    b_ent = np.asarray(inputs["b_ent"], np.float32)

    qmul = np.float32((1.0 / np.sqrt(np.float32(HD))) / 0.1)

    wq = w_qkv[:H] * ln_g[None, :]
    wk = w_qkv[H:2 * H] * ln_g[None, :]
    wv = w_qkv[2 * H:] * ln_g[None, :]
    bq = (b_qkv[:H] + wq @ ln_b) * qmul
    bk = b_qkv[H:2 * H] + wk @ ln_b
    bv = b_qkv[2 * H:] + wv @ ln_b
    wq = wq * qmul
    went = (w_ent * ln_g[None, :])[0]
    bent = np.float32(b_ent[0] + w_ent[0] @ ln_b)
    w_out = np.asarray(inputs["w_out"], np.float32)

    ident = np.eye(128, dtype=np.float32)
    umask = np.ascontiguousarray(np.broadcast_to(
        np.triu(np.ones((128, 128), np.float32))[:, None, :],
        (128, HPC, 128)).reshape(128, HPC * 128))

    in_maps = []
    for c in range(NCORES):
        b, g = divmod(c, NCORES // B)
        r = slice(g * HPC * HD, (g + 1) * HPC * HD)
        wqkT = np.concatenate([wq[r], wk[r]], axis=0).T       # [H, 512]
        wqk_r = np.ascontiguousarray(
            wqkT.reshape(KC, 128, 512).transpose(1, 0, 2).reshape(128, -1))
        wvgT = np.concatenate([wv[r], went[None, :],
                               np.zeros((1, H), np.float32)], axis=0).T
        wvg_r = np.ascontiguousarray(
            wvgT.reshape(KC, 128, 258).transpose(1, 0, 2).reshape(128, -1))
        woT = (0.1 * w_out[:, r]).T                           # [256, H]
        wo_r = np.ascontiguousarray(
            woT.reshape(2, 128, H).transpose(1, 0, 2).reshape(128, -1))
        qkb_r = np.ascontiguousarray(
            np.concatenate([bq[r], bk[r]]).reshape(4, 128).T)
        vb_r = np.ascontiguousarray(
            np.broadcast_to(bv[r][None, :], (128, 256)))
        in_maps.append({
            "x": x[b], "wqk": wqk_r, "wvg": wvg_r, "wo": wo_r,
            "qkb": qkb_r, "vb": vb_r,
            "negentb": np.full((128, 1), -bent, np.float32),
            "ident": ident, "umask": umask,
        })
    return in_maps


def _unshard(inputs, results):
    b_out = np.asarray(inputs["b_out"], np.float32)
    outs = []
    for b in range(B):
        g0 = b * (NCORES // B)
        acc = results[g0]["out_part"].astype(np.float32)
        for g in range(g0 + 1, g0 + NCORES // B):
            acc = acc + results[g]["out_part"]
        outs.append(acc + 0.1 * b_out[None, :])
    return np.stack(outs)


def run(inputs, **kw):
    nc = _get_nc()
    res = run_bass_kernel_spmd(nc, _in_maps(inputs),
                               core_ids=list(range(NCORES)), **kw)
    return _unshard(inputs, res.results), res


def kernel(**inputs) -> np.ndarray:
    out, _ = run(inputs)
    return out
